# revision 19
# baseline (speedup 1.0000x reference)
"""Trainium2 Bass kernel for nn_Llama_head (paired two-tower MLP head).

Computes sigmoid(rowwise_dot(mlp_u(xu), mlp_i(xv))) for N=32768 rows,
data-parallel across 8 NeuronCores (N sharded, weights replicated).

Per-core dataflow (Nc = 4096 rows), j-pair pipelined (256 rows / step):
  1. SWDGE cast-DMA streams x as [128, 4096] bf16 j-slices into a deep
     (14-buf) SBUF FIFO; first slices are chunked for a fast PE start.
  2. Per (tower, j-pair): PE transposes 128x128 tiles into PSUM in
     k-pair batches [128, 512]; DVE/ACT alternate copying them to SBUF.
  3. L1: ph[h, n] accumulated over 32 k-tiles as 256-col matmuls, both
     h-halves sharing one PSUM bank / accumulation group.
  4. ACT: h = relu(ph + b1) -> bf16; L2: puv[64, n] = w2.T @ h.
  5. DVE: u = puv + b2; prod = u * v; PE: ones.T @ prod -> diag[1, n];
     ACT sigmoid; one small out-DMA per j-pair on the SP queue.
"""

import os

import numpy as np
import ml_dtypes

# Problem shape (hardcoded per harness contract).
N_FULL = 32768
D = 4096
H = 256
O = 64
N_CORES = 8

NC_ROWS = N_FULL // N_CORES  # rows per core
TRACE = bool(int(os.environ.get("KERNEL_TRACE", "0")))

LAST_RESULTS = None  # BassKernelResults of the most recent run (for profiling)

_PROGRAM = None


def _build_program():
    from contextlib import ExitStack

    import concourse.mybir as mybir
    import concourse.tile as tile
    from concourse import bacc

    f32 = mybir.dt.float32
    bf16 = mybir.dt.bfloat16
    AF = mybir.ActivationFunctionType

    n_rows = NC_ROWS
    kt = D // 128              # 32 k-tiles along the contraction dim
    n_slices = n_rows // 128   # 32 j-slices per tower
    n_jp = n_slices // 2       # 16 j-pairs (256 rows each)
    NB = 256                   # rows per pipeline step

    nc = bacc.Bacc("TRN2")

    xu = nc.dram_tensor("xu", [n_rows, D], f32, kind="ExternalInput")
    xv = nc.dram_tensor("xv", [n_rows, D], f32, kind="ExternalInput")
    w1u = nc.dram_tensor("w1u", [D, H], bf16, kind="ExternalInput")
    w1i = nc.dram_tensor("w1i", [D, H], bf16, kind="ExternalInput")
    w2u = nc.dram_tensor("w2u", [H, O], bf16, kind="ExternalInput")
    w2i = nc.dram_tensor("w2i", [H, O], bf16, kind="ExternalInput")
    # Packed small constants (biases f32; identity+ones bf16) — dense
    # partition-major layouts so their DMAs are cheap and fast.
    cst_d = nc.dram_tensor("cst", [128, 6], f32, kind="ExternalInput")
    identp_d = nc.dram_tensor("identp", [128, 129], bf16, kind="ExternalInput")
    out = nc.dram_tensor("out", [n_rows], f32, kind="ExternalOutput")

    with ExitStack() as ctx:
        tc = ctx.enter_context(tile.TileContext(nc))

        wpool = ctx.enter_context(tc.tile_pool(name="weights", bufs=1))
        natp = ctx.enter_context(tc.tile_pool(name="nat", bufs=14))
        xtp = ctx.enter_context(tc.tile_pool(name="xt", bufs=8))
        hp = ctx.enter_context(tc.tile_pool(name="h", bufs=3))
        uvp = ctx.enter_context(tc.tile_pool(name="uv", bufs=6))
        sp = ctx.enter_context(tc.tile_pool(name="sacc", bufs=2))
        ps_h = ctx.enter_context(tc.tile_pool(name="psh", bufs=2, space="PSUM"))
        ps_t = ctx.enter_context(tc.tile_pool(name="pst", bufs=4, space="PSUM"))
        ps_uv = ctx.enter_context(tc.tile_pool(name="psuv", bufs=1, space="PSUM"))
        ps_d = ctx.enter_context(tc.tile_pool(name="psd", bufs=1, space="PSUM"))

        # cst rides the (otherwise idle) sync HWDGE queue and is available
        # almost immediately. identp is intentionally NOT loaded here: it is
        # placed mid-way into the SWDGE startup stream below, which gates
        # the first PE transpose until ~2.5MB of x has landed — the PE then
        # starts with a buffer and runs gaplessly (its p-state never drops).
        cst = wpool.tile([128, 6], f32, tag="cst", name="cst")
        nc.sync.dma_start(cst, cst_d[:])
        identp = wpool.tile([128, 129], bf16, tag="identp", name="identp")
        ident = identp[:, 0:128]
        ones = identp[:O, 128:129]
        b1_sb = {"u": cst[:, 0:2], "i": cst[:, 2:4]}
        b2_sb = {"u": cst[:O, 4:5], "i": cst[:O, 5:6]}

        # x views: slice j of a tower = rows [j*128, (j+1)*128), natural
        # layout [128 rows(part), 4096 d(free)], 16KB/partition reads.
        x_res = {
            "u": xu.rearrange("(j p) d -> j p d", p=128),
            "i": xv.rearrange("(j p) d -> j p d", p=128),
        }
        w1_d = {"u": w1u, "i": w1i}
        w2_d = {"u": w2u, "i": w2i}

        w1_sb = {}
        w2_sb = {}
        nat_tiles = {}  # (tower, j) -> sbuf tile

        def issue_slice_dma(tower, j, eng=None, chunks=1):
            eng = eng or nc.gpsimd
            nat = natp.tile([128, D], bf16, tag="nat", name="nat")
            cw = D // chunks
            for c in range(chunks):
                eng.dma_start(
                    nat[:, c * cw : (c + 1) * cw],
                    x_res[tower][j][:, c * cw : (c + 1) * cw],
                )
            nat_tiles[(tower, j)] = nat

        def issue_w2(tower):
            w2_sb[tower] = wpool.tile(
                [128, H // 128, O], bf16, tag=f"w2{tower}", name=f"w2{tower}"
            )
            nc.gpsimd.dma_start(
                w2_sb[tower], w2_d[tower].rearrange("(a p) o -> p a o", p=128)
            )

        # --- DMA issue order (one SWDGE FIFO = arrival order) ---
        # Startup is supply-paced: the first j-pair of each tower arrives
        # in k-consumption order — x column-chunks aligned to the kp loop,
        # with the matching w1 k-range chunks interleaved — so the PE's
        # warm-up never waits on data that is queued behind bytes it does
        # not need yet. w2 (tiny) lands before the first epilogue.
        def first_pair_chunked(tower, jp, cuts, wk, gate_after=None):
            nat0 = natp.tile([128, D], bf16, tag="nat", name="nat")
            nat1 = natp.tile([128, D], bf16, tag="nat", name="nat")
            nat_tiles[(tower, 2 * jp)] = nat0
            nat_tiles[(tower, 2 * jp + 1)] = nat1
            j0, j1 = 2 * jp, 2 * jp + 1
            for ci, ((c0, c1), (ka, kb)) in enumerate(
                zip(zip(cuts[:-1], cuts[1:]), wk)
            ):
                nc.gpsimd.dma_start(nat0[:, c0:c1], x_res[tower][j0][:, c0:c1])
                nc.gpsimd.dma_start(nat1[:, c0:c1], x_res[tower][j1][:, c0:c1])
                if tower not in w1_sb:
                    w1_sb[tower] = wpool.tile(
                        [128, kt, H], bf16, tag=f"w1{tower}", name=f"w1{tower}"
                    )
                src = w1_d[tower].rearrange("(k p) h -> p k h", p=128)
                nc.gpsimd.dma_start(w1_sb[tower][:, ka:kb], src[:, ka:kb])
                if ci == gate_after:
                    # PE start gate: every transpose reads ident, so this
                    # placement releases the PE only once everything queued
                    # above has landed.
                    nc.gpsimd.dma_start(identp, identp_d[:])

        first_pair_chunked(
            "u", 0, [0, 512, 1024, 2048, D],
            [(0, 4), (4, 8), (8, 16), (16, kt)], gate_after=2,
        )
        first_pair_chunked(
            "i", 0, [0, 512, 1024, 2048, D],
            [(0, 4), (4, 8), (8, 16), (16, kt)],
        )
        issue_w2("u")
        issue_w2("i")
        issue_slice_dma("u", 2)
        issue_slice_dma("u", 3)
        issue_slice_dma("i", 2)
        issue_slice_dma("i", 3)
        for jp in range(2, n_jp):
            for tower in ("u", "i"):
                issue_slice_dma(tower, 2 * jp)
                issue_slice_dma(tower, 2 * jp + 1)

        # --- main pipeline ---
        # One "step" = one (tower, j-pair). Within a step the k-pair loop is
        # software-pipelined depth-2 (transposes + copies run two k-pairs
        # ahead of the L1 matmuls) so the PE never waits on the PSUM->SBUF
        # copy round trip. The previous step's epilogue (relu, L2, bias,
        # and the j-pair finisher) is emitted inside the warm-up bubble of
        # the current step's k-loop, keeping the PE stream gapless.
        from collections import deque

        stash = {}

        def epilogue_relu(tower, ph):
            hsb = hp.tile([128, 2 * NB], bf16, tag="h", name="h")
            for hh in range(2):
                nc.scalar.activation(
                    hsb[:, hh * NB : (hh + 1) * NB],
                    ph[:, hh * NB : (hh + 1) * NB],
                    AF.Relu,
                    bias=b1_sb[tower][:, hh : hh + 1],
                )
            return hsb

        def epilogue(tower, jp, hsb):
            puv = ps_uv.tile([O, NB], f32, tag="puv", name="puv")
            for hh in range(2):
                nc.tensor.matmul(
                    puv,
                    w2_sb[tower][:, hh, :],
                    hsb[:, hh * NB : (hh + 1) * NB],
                    start=(hh == 0),
                    stop=(hh == 1),
                )
            usb = uvp.tile([O, NB], bf16, tag="uv", name="uv")
            nc.vector.tensor_scalar_add(usb, puv, b2_sb[tower])
            stash[tower] = usb
            if tower == "i":
                prod = uvp.tile([O, NB], bf16, tag="prod", name="prod")
                nc.vector.tensor_mul(prod, stash["u"], stash["i"])
                ps = ps_d.tile([1, NB], f32, tag="diag", name="ps")
                nc.tensor.matmul(ps, ones, prod, start=True, stop=True)
                s_blk = sp.tile([1, NB], f32, tag="sblk", name="s_blk")
                nc.scalar.activation(s_blk, ps, AF.Sigmoid)
                nc.sync.dma_start(out[jp * NB : (jp + 1) * NB], s_blk)

        prev = None  # (tower, jp, ph) awaiting epilogue
        for jp in range(n_jp):
            for tower in ("u", "i"):
                nat0 = nat_tiles[(tower, 2 * jp)]
                nat1 = nat_tiles[(tower, 2 * jp + 1)]

                # Both h-halves accumulate in one PSUM bank / zero-region
                # group: start only on the very first matmul, stop on the
                # very last.
                ph = ps_h.tile([128, 2 * NB], f32, tag="ph", name="ph")

                def transpose_and_copy(kp, flip):
                    pst = ps_t.tile([128, 512], bf16, tag="pst", name="pst")
                    for g in range(2):
                        k = 2 * kp + g
                        nc.tensor.transpose(
                            pst[:, g * 256 : g * 256 + 128],
                            nat0[:, k * 128 : (k + 1) * 128],
                            ident,
                        )
                        nc.tensor.transpose(
                            pst[:, g * 256 + 128 : g * 256 + 256],
                            nat1[:, k * 128 : (k + 1) * 128],
                            ident,
                        )
                    xt = xtp.tile([128, 512], bf16, tag="xt", name="xt")
                    # Alternate the PSUM->SBUF drain between DVE and ACT so
                    # neither engine sits on the PE's critical path.
                    if flip:
                        nc.vector.tensor_copy(xt, pst)
                    else:
                        nc.scalar.copy(xt, pst)
                    return xt

                def l1_mms(xt, kp, ph=ph, tower=tower):
                    for g in range(2):
                        k = 2 * kp + g
                        for hh in range(2):
                            nc.tensor.matmul(
                                ph[:, hh * NB : (hh + 1) * NB],
                                w1_sb[tower][:, k, hh * 128 : (hh + 1) * 128],
                                xt[:, g * 256 : (g + 1) * 256],
                                start=(k == 0 and hh == 0),
                                stop=(k == kt - 1 and hh == 1),
                            )

                pending = deque()
                for kp in range(kt // 2):
                    # 2:1 DVE:ACT copy split (DVE copies are faster; ACT
                    # keeps enough slack for the relus between copies).
                    xt = transpose_and_copy(kp, kp % 3 != 2)
                    pending.append((xt, kp))
                    if kp == 1 and prev is not None:
                        # The previous step's relu can start as soon as its
                        # PSUM accumulation closed; emit it early so the
                        # PE-side epilogue at kp==3 never waits on ACT.
                        prev = (prev[0], prev[1], epilogue_relu(prev[0], prev[2]))
                    if kp == 3 and prev is not None:
                        # Fill the pipeline warm-up bubble with the
                        # previous step's epilogue.
                        epilogue(*prev)
                        prev = None
                    if len(pending) > 3:
                        l1_mms(*pending.popleft())
                while pending:
                    l1_mms(*pending.popleft())
                prev = (tower, jp, ph)
        prev = (prev[0], prev[1], epilogue_relu(prev[0], prev[2]))
        epilogue(*prev)

    nc.compile()
    return nc


def _pack_cst(b1u, b1i, b2u, b2i):
    """[128, 6] f32: b1u as 2 cols, b1i as 2 cols, b2u, b2i (zero-padded)."""
    cst = np.zeros((128, 6), dtype=np.float32)
    cst[:, 0:2] = b1u.reshape(2, 128).T
    cst[:, 2:4] = b1i.reshape(2, 128).T
    cst[: b2u.shape[0], 4] = b2u
    cst[: b2i.shape[0], 5] = b2i
    return cst


def _pack_identp():
    """[128, 129] bf16: identity matrix plus a ones column."""
    p = np.zeros((128, 129), dtype=ml_dtypes.bfloat16)
    p[:, :128] = np.eye(128, dtype=ml_dtypes.bfloat16)
    p[:, 128] = 1
    return p


def _get_program():
    global _PROGRAM
    if _PROGRAM is None:
        _PROGRAM = _build_program()
    return _PROGRAM


def kernel(
    user_origin_emb,
    item_origin_emb,
    u_w1,
    u_b1,
    u_w2,
    u_b2,
    i_w1,
    i_b1,
    i_w2,
    i_b2,
):
    global LAST_RESULTS
    from concourse.bass_utils import run_bass_kernel_spmd

    xu = np.asarray(user_origin_emb, dtype=np.float32)
    xv = np.asarray(item_origin_emb, dtype=np.float32)
    shared = {
        "w1u": np.asarray(u_w1, dtype=np.float32).astype(ml_dtypes.bfloat16),
        "w1i": np.asarray(i_w1, dtype=np.float32).astype(ml_dtypes.bfloat16),
        "w2u": np.asarray(u_w2, dtype=np.float32).astype(ml_dtypes.bfloat16),
        "w2i": np.asarray(i_w2, dtype=np.float32).astype(ml_dtypes.bfloat16),
        "cst": _pack_cst(
            np.asarray(u_b1, dtype=np.float32),
            np.asarray(i_b1, dtype=np.float32),
            np.asarray(u_b2, dtype=np.float32),
            np.asarray(i_b2, dtype=np.float32),
        ),
        "identp": _pack_identp(),
    }

    nc = _get_program()
    n_rows = xu.shape[0] // N_CORES
    in_maps = [
        {
            "xu": xu[c * n_rows : (c + 1) * n_rows],
            "xv": xv[c * n_rows : (c + 1) * n_rows],
            **shared,
        }
        for c in range(N_CORES)
    ]
    res = run_bass_kernel_spmd(nc, in_maps, core_ids=list(range(N_CORES)), trace=TRACE)
    LAST_RESULTS = res
    return np.concatenate([r["out"] for r in res.results], axis=0)


# revision 22
# speedup vs baseline: 1.0600x; 1.0600x over previous
"""Trainium2 Bass kernel for nn_Llama_head (paired two-tower MLP head).

Computes sigmoid(rowwise_dot(mlp_u(xu), mlp_i(xv))) for N=32768 rows,
data-parallel across 8 NeuronCores (N sharded, weights replicated).

Per-core dataflow (Nc = 4096 rows), j-pair pipelined (256 rows / step):
  1. SWDGE cast-DMA streams x as [128, 4096] bf16 j-slices into a deep
     (14-buf) SBUF FIFO; first slices are chunked for a fast PE start.
  2. Per (tower, j-pair): PE transposes 128x128 tiles into PSUM in
     k-pair batches [128, 512]; DVE/ACT alternate copying them to SBUF.
  3. L1: ph[h, n] accumulated over 32 k-tiles as 256-col matmuls, both
     h-halves sharing one PSUM bank / accumulation group.
  4. ACT: h = relu(ph + b1) -> bf16; L2: puv[64, n] = w2.T @ h.
  5. DVE: u = puv + b2; prod = u * v; PE: ones.T @ prod -> diag[1, n];
     ACT sigmoid; one small out-DMA per j-pair on the SP queue.
"""

import os

import numpy as np
import ml_dtypes

# Problem shape (hardcoded per harness contract).
N_FULL = 32768
D = 4096
H = 256
O = 64
N_CORES = 8

NC_ROWS = N_FULL // N_CORES  # rows per core
TRACE = bool(int(os.environ.get("KERNEL_TRACE", "0")))

LAST_RESULTS = None  # BassKernelResults of the most recent run (for profiling)

_PROGRAM = None


def _build_program():
    from contextlib import ExitStack

    import concourse.mybir as mybir
    import concourse.tile as tile
    from concourse import bacc

    f32 = mybir.dt.float32
    bf16 = mybir.dt.bfloat16
    AF = mybir.ActivationFunctionType

    n_rows = NC_ROWS
    kt = D // 128              # 32 k-tiles along the contraction dim
    n_slices = n_rows // 128   # 32 j-slices per tower
    n_jp = n_slices // 2       # 16 j-pairs (256 rows each)
    NB = 256                   # rows per pipeline step

    nc = bacc.Bacc("TRN2")

    xu = nc.dram_tensor("xu", [n_rows, D], f32, kind="ExternalInput")
    xv = nc.dram_tensor("xv", [n_rows, D], f32, kind="ExternalInput")
    w1u = nc.dram_tensor("w1u", [D, H], bf16, kind="ExternalInput")
    w1i = nc.dram_tensor("w1i", [D, H], bf16, kind="ExternalInput")
    w2u = nc.dram_tensor("w2u", [H, O], bf16, kind="ExternalInput")
    w2i = nc.dram_tensor("w2i", [H, O], bf16, kind="ExternalInput")
    # Packed small constants (biases f32; identity+ones bf16) — dense
    # partition-major layouts so their DMAs are cheap and fast.
    cst_d = nc.dram_tensor("cst", [128, 6], f32, kind="ExternalInput")
    identp_d = nc.dram_tensor("identp", [128, 129], bf16, kind="ExternalInput")
    out = nc.dram_tensor("out", [n_rows], f32, kind="ExternalOutput")

    with ExitStack() as ctx:
        tc = ctx.enter_context(tile.TileContext(nc))

        wpool = ctx.enter_context(tc.tile_pool(name="weights", bufs=1))
        natp = ctx.enter_context(tc.tile_pool(name="nat", bufs=14))
        xtp = ctx.enter_context(tc.tile_pool(name="xt", bufs=8))
        hp = ctx.enter_context(tc.tile_pool(name="h", bufs=3))
        uvp = ctx.enter_context(tc.tile_pool(name="uv", bufs=6))
        sp = ctx.enter_context(tc.tile_pool(name="sacc", bufs=2))
        ps_h = ctx.enter_context(tc.tile_pool(name="psh", bufs=2, space="PSUM"))
        ps_t = ctx.enter_context(tc.tile_pool(name="pst", bufs=4, space="PSUM"))
        ps_uv = ctx.enter_context(tc.tile_pool(name="psuv", bufs=1, space="PSUM"))
        ps_d = ctx.enter_context(tc.tile_pool(name="psd", bufs=1, space="PSUM"))

        # Constants on the (otherwise idle) sync HWDGE queue: available
        # almost immediately, so the first transposes aren't gated. (Note:
        # only gpsimd/SWDGE can cast, so x must stay on gpsimd; and the
        # SWDGE startup is issue-cadence-bound at ~1us/instruction, so the
        # startup wants FEW instructions, in strict consumption order.)
        cst = wpool.tile([128, 6], f32, tag="cst", name="cst")
        nc.sync.dma_start(cst, cst_d[:])
        identp = wpool.tile([128, 129], bf16, tag="identp", name="identp")
        nc.sync.dma_start(identp, identp_d[:])
        ident = identp[:, 0:128]
        ones = identp[:O, 128:129]
        b1_sb = {"u": cst[:, 0:2], "i": cst[:, 2:4]}
        b2_sb = {"u": cst[:O, 4:5], "i": cst[:O, 5:6]}

        # x views: slice j of a tower = rows [j*128, (j+1)*128), natural
        # layout [128 rows(part), 4096 d(free)], 16KB/partition reads.
        x_res = {
            "u": xu.rearrange("(j p) d -> j p d", p=128),
            "i": xv.rearrange("(j p) d -> j p d", p=128),
        }
        w1_d = {"u": w1u, "i": w1i}
        w2_d = {"u": w2u, "i": w2i}

        w1_sb = {}
        w2_sb = {}
        nat_tiles = {}  # (tower, j) -> sbuf tile

        def issue_slice_dma(tower, j, eng=None, chunks=1):
            eng = eng or nc.gpsimd
            nat = natp.tile([128, D], bf16, tag="nat", name="nat")
            cw = D // chunks
            for c in range(chunks):
                eng.dma_start(
                    nat[:, c * cw : (c + 1) * cw],
                    x_res[tower][j][:, c * cw : (c + 1) * cw],
                )
            nat_tiles[(tower, j)] = nat

        def issue_w2(tower):
            w2_sb[tower] = wpool.tile(
                [128, H // 128, O], bf16, tag=f"w2{tower}", name=f"w2{tower}"
            )
            nc.gpsimd.dma_start(
                w2_sb[tower], w2_d[tower].rearrange("(a p) o -> p a o", p=128)
            )

        # --- DMA issue order (one SWDGE FIFO = arrival order) ---
        # Startup is supply-paced: the first j-pair of each tower arrives
        # in k-consumption order — x column-chunks aligned to the kp loop,
        # with the matching w1 k-range chunks interleaved — so the PE's
        # warm-up never waits on data that is queued behind bytes it does
        # not need yet. w2 (tiny) lands before the first epilogue.
        def first_pair_chunked(tower, jp, cuts, wk):
            nat0 = natp.tile([128, D], bf16, tag="nat", name="nat")
            nat1 = natp.tile([128, D], bf16, tag="nat", name="nat")
            nat_tiles[(tower, 2 * jp)] = nat0
            nat_tiles[(tower, 2 * jp + 1)] = nat1
            j0, j1 = 2 * jp, 2 * jp + 1
            for (c0, c1), (ka, kb) in zip(zip(cuts[:-1], cuts[1:]), wk):
                nc.gpsimd.dma_start(nat0[:, c0:c1], x_res[tower][j0][:, c0:c1])
                nc.gpsimd.dma_start(nat1[:, c0:c1], x_res[tower][j1][:, c0:c1])
                if tower not in w1_sb:
                    w1_sb[tower] = wpool.tile(
                        [128, kt, H], bf16, tag=f"w1{tower}", name=f"w1{tower}"
                    )
                src = w1_d[tower].rearrange("(k p) h -> p k h", p=128)
                nc.gpsimd.dma_start(w1_sb[tower][:, ka:kb], src[:, ka:kb])

        first_pair_chunked("u", 0, [0, 1024, 2048, D], [(0, 8), (8, 16), (16, kt)])
        first_pair_chunked("i", 0, [0, 1024, 2048, D], [(0, 8), (8, 16), (16, kt)])
        issue_w2("u")
        issue_w2("i")
        issue_slice_dma("u", 2)
        issue_slice_dma("u", 3)
        issue_slice_dma("i", 2)
        issue_slice_dma("i", 3)
        for jp in range(2, n_jp):
            for tower in ("u", "i"):
                issue_slice_dma(tower, 2 * jp)
                issue_slice_dma(tower, 2 * jp + 1)

        # --- main pipeline ---
        # One "step" = one (tower, j-pair). Within a step the k-pair loop is
        # software-pipelined depth-2 (transposes + copies run two k-pairs
        # ahead of the L1 matmuls) so the PE never waits on the PSUM->SBUF
        # copy round trip. The previous step's epilogue (relu, L2, bias,
        # and the j-pair finisher) is emitted inside the warm-up bubble of
        # the current step's k-loop, keeping the PE stream gapless.
        from collections import deque

        stash = {}

        def epilogue_relu(tower, ph):
            hsb = hp.tile([128, 2 * NB], bf16, tag="h", name="h")
            for hh in range(2):
                nc.scalar.activation(
                    hsb[:, hh * NB : (hh + 1) * NB],
                    ph[:, hh * NB : (hh + 1) * NB],
                    AF.Relu,
                    bias=b1_sb[tower][:, hh : hh + 1],
                )
            return hsb

        def epilogue(tower, jp, hsb):
            puv = ps_uv.tile([O, NB], f32, tag="puv", name="puv")
            for hh in range(2):
                nc.tensor.matmul(
                    puv,
                    w2_sb[tower][:, hh, :],
                    hsb[:, hh * NB : (hh + 1) * NB],
                    start=(hh == 0),
                    stop=(hh == 1),
                )
            usb = uvp.tile([O, NB], bf16, tag="uv", name="uv")
            nc.vector.tensor_scalar_add(usb, puv, b2_sb[tower])
            stash[tower] = usb
            if tower == "i":
                prod = uvp.tile([O, NB], bf16, tag="prod", name="prod")
                nc.vector.tensor_mul(prod, stash["u"], stash["i"])
                ps = ps_d.tile([1, NB], f32, tag="diag", name="ps")
                nc.tensor.matmul(ps, ones, prod, start=True, stop=True)
                s_blk = sp.tile([1, NB], f32, tag="sblk", name="s_blk")
                nc.scalar.activation(s_blk, ps, AF.Sigmoid)
                nc.sync.dma_start(out[jp * NB : (jp + 1) * NB], s_blk)

        prev = None  # (tower, jp, ph) awaiting epilogue
        for jp in range(n_jp):
            for tower in ("u", "i"):
                nat0 = nat_tiles[(tower, 2 * jp)]
                nat1 = nat_tiles[(tower, 2 * jp + 1)]

                # Both h-halves accumulate in one PSUM bank / zero-region
                # group: start only on the very first matmul, stop on the
                # very last.
                ph = ps_h.tile([128, 2 * NB], f32, tag="ph", name="ph")

                def transpose_and_copy(kp, flip):
                    pst = ps_t.tile([128, 512], bf16, tag="pst", name="pst")
                    for g in range(2):
                        k = 2 * kp + g
                        nc.tensor.transpose(
                            pst[:, g * 256 : g * 256 + 128],
                            nat0[:, k * 128 : (k + 1) * 128],
                            ident,
                        )
                        nc.tensor.transpose(
                            pst[:, g * 256 + 128 : g * 256 + 256],
                            nat1[:, k * 128 : (k + 1) * 128],
                            ident,
                        )
                    xt = xtp.tile([128, 512], bf16, tag="xt", name="xt")
                    # Alternate the PSUM->SBUF drain between DVE and ACT so
                    # neither engine sits on the PE's critical path.
                    if flip:
                        nc.vector.tensor_copy(xt, pst)
                    else:
                        nc.scalar.copy(xt, pst)
                    return xt

                def l1_mms(xt, kp, ph=ph, tower=tower):
                    for g in range(2):
                        k = 2 * kp + g
                        for hh in range(2):
                            nc.tensor.matmul(
                                ph[:, hh * NB : (hh + 1) * NB],
                                w1_sb[tower][:, k, hh * 128 : (hh + 1) * 128],
                                xt[:, g * 256 : (g + 1) * 256],
                                start=(k == 0 and hh == 0),
                                stop=(k == kt - 1 and hh == 1),
                            )

                pending = deque()
                for kp in range(kt // 2):
                    # 2:1 DVE:ACT copy split (DVE copies are faster; ACT
                    # keeps enough slack for the relus between copies).
                    xt = transpose_and_copy(kp, kp % 3 != 2)
                    pending.append((xt, kp))
                    if kp == 1 and prev is not None:
                        # The previous step's relu can start as soon as its
                        # PSUM accumulation closed; emit it early so the
                        # PE-side epilogue at kp==3 never waits on ACT.
                        prev = (prev[0], prev[1], epilogue_relu(prev[0], prev[2]))
                    if kp == 3 and prev is not None:
                        # Fill the pipeline warm-up bubble with the
                        # previous step's epilogue.
                        epilogue(*prev)
                        prev = None
                    if len(pending) > 2:
                        l1_mms(*pending.popleft())
                while pending:
                    l1_mms(*pending.popleft())
                prev = (tower, jp, ph)
        prev = (prev[0], prev[1], epilogue_relu(prev[0], prev[2]))
        epilogue(*prev)

    nc.compile()
    return nc


def _pack_cst(b1u, b1i, b2u, b2i):
    """[128, 6] f32: b1u as 2 cols, b1i as 2 cols, b2u, b2i (zero-padded)."""
    cst = np.zeros((128, 6), dtype=np.float32)
    cst[:, 0:2] = b1u.reshape(2, 128).T
    cst[:, 2:4] = b1i.reshape(2, 128).T
    cst[: b2u.shape[0], 4] = b2u
    cst[: b2i.shape[0], 5] = b2i
    return cst


def _pack_identp():
    """[128, 129] bf16: identity matrix plus a ones column."""
    p = np.zeros((128, 129), dtype=ml_dtypes.bfloat16)
    p[:, :128] = np.eye(128, dtype=ml_dtypes.bfloat16)
    p[:, 128] = 1
    return p


def _get_program():
    global _PROGRAM
    if _PROGRAM is None:
        _PROGRAM = _build_program()
    return _PROGRAM


def kernel(
    user_origin_emb,
    item_origin_emb,
    u_w1,
    u_b1,
    u_w2,
    u_b2,
    i_w1,
    i_b1,
    i_w2,
    i_b2,
):
    global LAST_RESULTS
    from concourse.bass_utils import run_bass_kernel_spmd

    xu = np.asarray(user_origin_emb, dtype=np.float32)
    xv = np.asarray(item_origin_emb, dtype=np.float32)
    shared = {
        "w1u": np.asarray(u_w1, dtype=np.float32).astype(ml_dtypes.bfloat16),
        "w1i": np.asarray(i_w1, dtype=np.float32).astype(ml_dtypes.bfloat16),
        "w2u": np.asarray(u_w2, dtype=np.float32).astype(ml_dtypes.bfloat16),
        "w2i": np.asarray(i_w2, dtype=np.float32).astype(ml_dtypes.bfloat16),
        "cst": _pack_cst(
            np.asarray(u_b1, dtype=np.float32),
            np.asarray(i_b1, dtype=np.float32),
            np.asarray(u_b2, dtype=np.float32),
            np.asarray(i_b2, dtype=np.float32),
        ),
        "identp": _pack_identp(),
    }

    nc = _get_program()
    n_rows = xu.shape[0] // N_CORES
    in_maps = [
        {
            "xu": xu[c * n_rows : (c + 1) * n_rows],
            "xv": xv[c * n_rows : (c + 1) * n_rows],
            **shared,
        }
        for c in range(N_CORES)
    ]
    res = run_bass_kernel_spmd(nc, in_maps, core_ids=list(range(N_CORES)), trace=TRACE)
    LAST_RESULTS = res
    return np.concatenate([r["out"] for r in res.results], axis=0)
